# revision 38
# baseline (speedup 1.0000x reference)
"""Trainium2 Bass kernel for the attention-LSTM captioning RNN.

Strategy: 8-way tensor-parallel on the 4H gate axis (each core owns 128
columns of each of the i/f/o/g gate blocks = 512 columns). Full batch N=128
on every core so the 128x128 PE array stays fully utilized.

Per step, the only cross-core exchange is one remote-DMA broadcast
(SBUF -> SBUF, all 8 cores on one chip, no HBM bounce / collective firmware)
of a 320B/partition payload: [h_t.T slice (bf16, 128 cols) | partial
attention scores (f32, 16)]. The attention-output GEMM (attn @ Wattn) is
eliminated by pre-folding Wattn through A:  B[n,p,c] = sum_h A[n,h,p] *
Wattn[h,c]  (computed once on device), so the per-step attention contribution
is z[n,c] = sum_p w[n,p] * B[n,p,c], evaluated as 16 block-diagonal matmuls
with the softmax weights as the stationary operand.

Softmax skips max-subtraction (scores are O(1) here); normalization is
deferred: z is accumulated unnormalized and divided by the per-sample
denominator during gate pre-activation assembly.

The entire per-step schedule runs inside per-engine Fori hardware loops with
register-tracked semaphore targets, so program size (and hence BIR->NEFF
compile and NEFF load time) is constant in T.
"""
import numpy as np
import ml_dtypes

import concourse.bass as bass
import concourse.mybir as mybir
from concourse import library_config
from concourse.bass_utils import run_bass_kernel_spmd
from concourse.library_overlay import lower_extended_insts

N, T, D, H = 128, 128, 1024, 1024
NCORES = 8
CPC = 512          # gate columns per core
PAYB = 320         # payload bytes/partition: 256 (128 bf16 hT) + 64 (16 f32 pscores)
BF16 = mybir.dt.bfloat16
F32 = mybir.dt.float32
F32R = mybir.dt.float32r
U8 = mybir.dt.uint8
I32 = mybir.dt.int32
BF = ml_dtypes.bfloat16


def build(t_steps=T, xf=None, of=None, skip=()):
    if xf is None:
        xf = t_steps
    if of is None:
        of = t_steps
    assert t_steps % 2 == 0
    nc = bass.Bass(detect_race_conditions=False)
    ddp = nc.declare_dram_parameter
    xtb = ddp("xtb", [xf, 128, D], BF16, isOutput=False)  # pre-transposed x
    wh = ddp("wh", [128, 8 * CPC], BF16, isOutput=False)
    wx = ddp("wx", [128, 8 * CPC], BF16, isOutput=False)
    b4i = ddp("b4i", [128, 16 * 512], BF16, isOutput=False)  # host-folded Wattn@A
    h0t = ddp("h0t", [128, 128], F32, isOutput=False)        # h0 slice transposed
    c0i = ddp("c0i", [128, 128], F32, isOutput=False)        # c0 = h0 slice
    a4tsc = ddp("a4tsc", [128, 2048], BF16, isOutput=False)
    btile = ddp("btile", [128, CPC], F32, isOutput=False)
    m512 = ddp("m512", [128, 512], BF16, isOutput=False)
    dm512 = ddp("dm512", [128, 512], F32, isOutput=False)
    ident = ddp("ident", [128, 128], F32, isOutput=False)
    onesb = ddp("onesb", [128, 1], BF16, isOutput=False)
    cid = ddp("cid", [1, 1], I32, isOutput=False)
    yout = ddp("yout", [N, of, 128], F32, isOutput=True)

    sb = nc.alloc_sbuf_tensor
    wh_s = sb("wh_s", [128, 8 * CPC], BF16)
    wx_s = sb("wx_s", [128, 8 * CPC], BF16)
    a4tsc_s = sb("a4tsc_s", [128, 2048], BF16)
    h0t_s = sb("h0t_s", [128, 128], F32)
    b4 = sb("b4", [128, 16 * 512], BF16)
    btile_s = sb("btile_s", [128, CPC], F32)
    m512_s = sb("m512_s", [128, 512], BF16)
    dm512_s = sb("dm512_s", [128, 512], F32)
    ident_s = sb("ident_s", [128, 128], F32)
    ones_s = sb("ones_s", [128, 1], BF16)
    cid_s = sb("cid_s", [1, 1], I32)
    xT = [sb(f"xT{p}", [128, 8 * 128], BF16) for p in range(2)]
    payload = [sb(f"payload{p}", [128, PAYB], U8) for p in range(2)]
    slots = [sb(f"slots{p}", [128, NCORES * PAYB], U8) for p in range(2)]
    s4 = sb("s4", [128, 16], F32)
    e4 = sb("e4", [128, 16], F32)
    bd4 = sb("bd4", [128, 512], BF16)
    recip = sb("recip", [128, 1], F32)
    acc = sb("acc", [128, CPC], F32)
    a_sb = sb("a_sb", [128, CPC], F32)
    gates = sb("gates", [128, CPC], F32)
    ctile = sb("ctile", [128, 128], F32)
    tmp1 = sb("tmp1", [128, 128], F32)
    tmp2 = sb("tmp2", [128, 128], F32)
    tanhc = sb("tanhc", [128, 128], F32)
    nh = [sb(f"nh{p}", [128, 128], F32) for p in range(2)]
    mskd = sb("mskd", [128, 512], F32)

    pA = [nc.alloc_psum_tensor(f"pA{p}", [128, 512], F32) for p in range(2)]
    pZ = nc.alloc_psum_tensor("pZ", [128, 512], F32)
    pS = nc.alloc_psum_tensor("pS", [128, 512], F32)
    pT = nc.alloc_psum_tensor("pT", [128, 128], F32)
    pD = nc.alloc_psum_tensor("pD", [128, 8], F32)

    def pay_ht(p):
        return payload[p][:, 0:256].bitcast(BF16)

    def pay_ps(p):
        return payload[p][:, 256:320].bitcast(F32)

    def slot_ht(p, j):
        return slots[p][:, j * PAYB: j * PAYB + 256].bitcast(BF16)

    rdests = [(0, j) for j in range(NCORES)]

    from contextlib import ExitStack
    _es = ExitStack()
    block = _es.enter_context(nc.Block())
    s_ld = _es.enter_context(nc.semaphore("s_ld"))
    s_x = _es.enter_context(nc.semaphore("s_x"))
    s_prep = _es.enter_context(nc.semaphore("s_prep"))
    # per-parity arrival/sent semaphores: a step-(t+2) frame (same parity)
    # can only exist after every core consumed step t, so counting per
    # parity makes the 16*(u+1) thresholds unambiguous.
    s_sent = [_es.enter_context(nc.semaphore(f"s_sent{p}")) for p in range(2)]
    s_arr = [_es.enter_context(nc.semaphore(f"s_arr{p}")) for p in range(2)]
    s_sum = _es.enter_context(nc.semaphore("s_sum"))
    s_exp = _es.enter_context(nc.semaphore("s_exp"))
    s_bd = _es.enter_context(nc.semaphore("s_bd"))
    s_ga = _es.enter_context(nc.semaphore("s_ga"))
    s_z = _es.enter_context(nc.semaphore("s_z"))
    s_zz = _es.enter_context(nc.semaphore("s_zz"))
    s_aa = _es.enter_context(nc.semaphore("s_aa"))
    s_sg = _es.enter_context(nc.semaphore("s_sg"))
    s_cu = _es.enter_context(nc.semaphore("s_cu"))
    s_th = _es.enter_context(nc.semaphore("s_th"))
    s_nh = _es.enter_context(nc.semaphore("s_nh"))
    s_nT = _es.enter_context(nc.semaphore("s_nT"))
    s_ph = _es.enter_context(nc.semaphore("s_ph"))
    s_ps = _es.enter_context(nc.semaphore("s_ps"))
    s_pp = _es.enter_context(nc.semaphore("s_pp"))
    s_out = _es.enter_context(nc.semaphore("s_out"))

    # -------------------- GPSIMD: all DMA + remote broadcast --------------
    @block.gpsimd
    def _(g):
        g.load_library(library_config.remote_dma)
        loads = [
            (wh_s, wh), (wx_s, wx), (b4, b4i),
            (a4tsc_s, a4tsc), (btile_s, btile), (ctile, c0i),
            (h0t_s, h0t), (m512_s, m512), (dm512_s, dm512),
            (ident_s, ident), (ones_s, onesb), (cid_s, cid),
        ]
        for dst, src in loads:
            g.dma_start(out=dst[:], in_=src[:]).then_inc(s_ld, 16)
        with (
            g.register("rk") as rk,
            g.register("r_pp") as r_pp,
            g.register("r_prep") as r_prep,
        ):
            g.wait_ge(s_ld, 16 * 12)
            g.reg_load(rk, cid_s[:1, :1])
            k_sv = g.snap(rk, min_val=0, max_val=NCORES - 1)
            g.reg_mov(r_pp, 0)
            g.reg_mov(r_prep, 0)
            with g.Fori(0, t_steps // 2):
                for p in range(2):
                    # prep this step's broadcast descriptor early
                    if "bc" in skip:
                        g.reg_add(r_pp, r_pp, 1)
                        g.wait_ge(s_pp, r_pp)
                        g.sem_inc(s_arr[p], 16)
                        g.sem_inc(s_sent[p], 16)
                    else:
                        g.remote_dma_broadcast(
                            out_ap=slots[p][:, bass.ts(k_sv, PAYB)],
                            in_ap=payload[p][:],
                            remote_sem=s_arr[p],
                            local_sem=s_sent[p],
                            rdests=rdests,
                        ).then_inc(s_prep, 1)
                        g.reg_add(r_prep, r_prep, 1)
                        g.reg_add(r_pp, r_pp, 1)
                        g.wait_ge(s_pp, r_pp)      # payload staged by DVE
                        g.wait_ge(s_prep, r_prep)  # descriptor written
                        g.trigger_dma(1)

    # ------------- SYNC (SP/HWDGE): x prefetch + y store ------------------
    @block.sync
    def _(sy):
        sy.dma_start(out=xT[0][:], in_=xtb[0, :, :]).then_inc(s_x, 16)
        with (
            sy.register("r_pp") as r_pp,
            sy.register("r_nh") as r_nh,
            sy.register("r_xi") as r_xi,
            sy.register("r_yi") as r_yi,
        ):
            sy.reg_mov(r_pp, 0)
            sy.reg_mov(r_nh, 0)
            sy.reg_mov(r_xi, 1 % xf)
            sy.reg_mov(r_yi, 0)
            with sy.Fori(0, t_steps // 2):
                for p in range(2):
                    # payload(t) staged implies PE finished Wx(t-1), which
                    # gates overwriting xT[(t+1)%2]
                    sy.reg_add(r_pp, r_pp, 1)
                    sy.wait_ge(s_pp, r_pp)
                    if "xy" in skip:
                        sy.sem_inc(s_x, 16)
                        sy.reg_add(r_nh, r_nh, 1)
                        sy.wait_ge(s_nh, r_nh)
                        sy.sem_inc(s_out, 16)
                        continue
                    # snap fresh each iteration — snap materializes the
                    # register value at snap point
                    xi_sv = sy.snap(r_xi, min_val=0, max_val=xf - 1)
                    sy.dma_start(
                        out=xT[1 - p][:], in_=xtb[bass.ds(xi_sv, 1), :, :]
                    ).then_inc(s_x, 16)
                    sy.reg_add(r_xi, r_xi, 1)
                    sy.reg_mod(r_xi, r_xi, xf)
                    # store output row t
                    sy.reg_add(r_nh, r_nh, 1)
                    sy.wait_ge(s_nh, r_nh)
                    yi_sv = sy.snap(r_yi, min_val=0, max_val=of - 1)
                    sy.dma_start(
                        out=yout[:, bass.ds(yi_sv, 1), :], in_=nh[p][:]
                    ).then_inc(s_out, 16)
                    sy.reg_add(r_yi, r_yi, 1)
                    sy.reg_mod(r_yi, r_yi, of)
            sy.wait_ge(s_out, 16 * t_steps)

    # -------------------- PE: tensor engine --------------------
    @block.tensor
    def _(te):
        te.wait_ge(s_ld, 16 * 12)
        # scores for payload(0)
        te.wait_ge(s_ph, 1)
        for b in range(16):
            gq = b // 4
            ins = te.matmul(
                pS[:, 32 * b: 32 * (b + 1)],
                a4tsc_s[:, 128 * b: 128 * (b + 1)],
                pay_ht(0)[:, 32 * gq: 32 * (gq + 1)],
                start=True, stop=True,
            )
        ins.then_inc(s_ps, 1)

        with (
            te.register("r_x") as r_x,
            te.register("r_arr0") as r_arr0,
            te.register("r_arr1") as r_arr1,
            te.register("r_bd") as r_bd,
            te.register("r_nh") as r_nh,
            te.register("r_ph") as r_ph,
        ):
            te.reg_mov(r_x, 0)
            te.reg_mov(r_arr0, 0)
            te.reg_mov(r_arr1, 0)
            te.reg_mov(r_bd, 0)
            te.reg_mov(r_nh, 0)
            te.reg_mov(r_ph, 1)
            with te.Fori(0, t_steps // 2):
                for p in range(2):
                    # GEMM: x_t @ Wx
                    te.reg_add(r_x, r_x, 16)
                    te.wait_ge(s_x, r_x)
                    for hc in range(8):
                        te.matmul(
                            pA[p][:, :],
                            xT[p][:, 128 * hc: 128 * (hc + 1)],
                            wx_s[:, 512 * hc: 512 * (hc + 1)],
                            start=(hc == 0), stop=False,
                        )
                    # GEMM: h_t @ Wh (stationary = gathered h.T slices)
                    r_arr = r_arr0 if p == 0 else r_arr1
                    te.reg_add(r_arr, r_arr, 16)
                    te.wait_ge(s_arr[p], r_arr)
                    for hc in range(8):
                        ins = te.matmul(
                            pA[p][:, :],
                            slot_ht(p, hc),
                            wh_s[:, 512 * hc: 512 * (hc + 1)],
                            start=False, stop=(hc == 7),
                        )
                    ins.then_inc(s_ga, 1)
                    # denominators first (recip overlaps z matmuls), then z
                    te.reg_add(r_bd, r_bd, 1)
                    te.wait_ge(s_bd, r_bd)
                    for g_ in range(4):
                        tp = {"tile_position": (0, 32 * g_)} if g_ == 3 else {}
                        for q in range(4):
                            b = 4 * g_ + q
                            ins = te.matmul(
                                pD[32 * g_: 32 * (g_ + 1), 0:1],
                                bd4[:, 32 * b: 32 * (b + 1)],
                                ones_s[:],
                                start=(q == 0), stop=(q == 3), **tp,
                            )
                    ins.then_inc(s_z, 1)
                    for g_ in range(4):
                        tp = {"tile_position": (0, 32 * g_)} if g_ == 3 else {}
                        for q in range(4):
                            b = 4 * g_ + q
                            ins = te.matmul(
                                pZ[32 * g_: 32 * (g_ + 1), :],
                                bd4[:, 32 * b: 32 * (b + 1)],
                                b4[:, 512 * b: 512 * (b + 1)],
                                start=(q == 0), stop=(q == 3), **tp,
                            )
                    ins.then_inc(s_zz, 1)
                    # nh.T
                    te.reg_add(r_nh, r_nh, 1)
                    te.wait_ge(s_nh, r_nh)
                    te.transpose(pT[:, :], nh[p][:], ident_s[:]).then_inc(s_nT, 1)
                    # partial scores for payload(t+1)
                    te.reg_add(r_ph, r_ph, 1)
                    te.wait_ge(s_ph, r_ph)
                    for b in range(16):
                        gq = b // 4
                        ins = te.matmul(
                            pS[:, 32 * b: 32 * (b + 1)],
                            a4tsc_s[:, 128 * b: 128 * (b + 1)],
                            pay_ht(1 - p)[:, 32 * gq: 32 * (gq + 1)],
                            start=True, stop=True,
                        )
                    ins.then_inc(s_ps, 1)

    # -------------------- DVE: vector engine --------------------
    @block.vector
    def _(v):
        v.wait_ge(s_ld, 16 * 12)
        # payload(0).hT = h0.T slice (cast f32 -> bf16)
        v.tensor_copy(pay_ht(0), h0t_s[:]).then_inc(s_ph, 1)
        # pscores(0) extract
        v.wait_ge(s_ps, 1)
        v.tensor_tensor(mskd[:], pS[:, :], dm512_s[:], mybir.AluOpType.mult)
        v.tensor_reduce(
            pay_ps(0),
            mskd[:].rearrange("p (b n) -> p b n", b=16),
            mybir.AxisListType.X, mybir.AluOpType.add,
        ).then_inc(s_pp, 1)

        slots_f32 = [slots[p][:].bitcast(F32) for p in range(2)]
        with (
            v.register("r_arr0") as r_arr0,
            v.register("r_arr1") as r_arr1,
            v.register("r_exp") as r_exp,
            v.register("r_z") as r_z,
            v.register("r_zz") as r_zz,
            v.register("r_ga") as r_ga,
            v.register("r_sg") as r_sg,
            v.register("r_th") as r_th,
            v.register("r_nT") as r_nT,
            v.register("r_s0") as r_s0,
            v.register("r_s1") as r_s1,
            v.register("r_ps") as r_ps,
        ):
            for r in (r_arr0, r_arr1, r_exp, r_z, r_zz, r_ga, r_sg, r_th, r_s0, r_s1):
                v.reg_mov(r, 0)
            v.reg_mov(r_nT, 0)  # no preamble transpose: s_nT counts loop only
            v.reg_mov(r_ps, 1)
            with v.Fori(0, t_steps // 2):
                for p in range(2):
                    # sum partial scores over the 8 slots
                    r_arr = r_arr0 if p == 0 else r_arr1
                    v.reg_add(r_arr, r_arr, 16)
                    v.wait_ge(s_arr[p], r_arr)
                    v.tensor_reduce(
                        s4[:],
                        slots_f32[p][:, :].rearrange(
                            "p (s c) -> p c s", s=NCORES
                        )[:, 64:80, :],
                        mybir.AxisListType.X, mybir.AluOpType.add,
                    ).then_inc(s_sum, 1)
                    # BD4 = mask * exp (broadcast E4 over 32 cols)
                    v.reg_add(r_exp, r_exp, 1)
                    v.wait_ge(s_exp, r_exp)
                    v.tensor_tensor(
                        bd4[:].rearrange("p (b n) -> p b n", b=16),
                        m512_s[:].rearrange("p (b n) -> p b n", b=16),
                        e4[:].rearrange("p (b u) -> p b u", u=1).broadcast_to([128, 16, 32]),
                        mybir.AluOpType.mult,
                    ).then_inc(s_bd, 1)
                    # recip of denominators (overlaps PE z matmuls)
                    v.reg_add(r_z, r_z, 1)
                    v.wait_ge(s_z, r_z)
                    v.reciprocal(recip[:], pD[:, 0:1])
                    # acc = pA + b as soon as the gate GEMMs finish
                    v.reg_add(r_ga, r_ga, 1)
                    v.wait_ge(s_ga, r_ga)
                    v.tensor_tensor(acc[:], pA[p][:, :], btile_s[:], mybir.AluOpType.add)
                    # a = (pZ * recip) + acc once z lands
                    v.reg_add(r_zz, r_zz, 1)
                    v.wait_ge(s_zz, r_zz)
                    v.scalar_tensor_tensor(
                        a_sb[:], pZ[:, :], recip[:], acc[:],
                        mybir.AluOpType.mult, mybir.AluOpType.add,
                    ).then_inc(s_aa, 1)
                    # c update
                    v.reg_add(r_sg, r_sg, 1)
                    v.wait_ge(s_sg, r_sg)
                    v.tensor_tensor(tmp1[:], gates[:, 0:128], gates[:, 384:512], mybir.AluOpType.mult)
                    v.tensor_tensor(tmp2[:], gates[:, 128:256], ctile[:], mybir.AluOpType.mult)
                    v.tensor_tensor(ctile[:], tmp1[:], tmp2[:], mybir.AluOpType.add).then_inc(s_cu, 1)
                    # nh = o * tanh(c)
                    v.reg_add(r_th, r_th, 1)
                    v.wait_ge(s_th, r_th)
                    v.tensor_tensor(nh[p][:], gates[:, 256:384], tanhc[:], mybir.AluOpType.mult).then_inc(s_nh, 1)
                    # payload(t+1).hT  (WAR: wait send of payload(t-1) done;
                    # t-1 has parity 1-p, its sends count on s_sent[1-p])
                    v.reg_add(r_nT, r_nT, 1)
                    v.wait_ge(s_nT, r_nT)
                    if p == 0:
                        v.wait_ge(s_sent[1], r_s1)
                        v.reg_add(r_s1, r_s1, 16)
                    else:
                        v.reg_add(r_s0, r_s0, 16)
                        v.wait_ge(s_sent[0], r_s0)
                    v.tensor_copy(pay_ht(1 - p), pT[:, :]).then_inc(s_ph, 1)
                    # pscores(t+1) extract
                    v.reg_add(r_ps, r_ps, 1)
                    v.wait_ge(s_ps, r_ps)
                    v.tensor_tensor(mskd[:], pS[:, :], dm512_s[:], mybir.AluOpType.mult)
                    v.tensor_reduce(
                        pay_ps(1 - p),
                        mskd[:].rearrange("p (b n) -> p b n", b=16),
                        mybir.AxisListType.X, mybir.AluOpType.add,
                    ).then_inc(s_pp, 1)

    # -------------------- ACT: scalar engine --------------------
    @block.scalar
    def _(sc):
        with (
            sc.register("r_sum") as r_sum,
            sc.register("r_aa") as r_aa,
            sc.register("r_cu") as r_cu,
        ):
            sc.reg_mov(r_sum, 0)
            sc.reg_mov(r_aa, 0)
            sc.reg_mov(r_cu, 0)
            with sc.Fori(0, t_steps // 2):
                for p in range(2):
                    sc.reg_add(r_sum, r_sum, 1)
                    sc.wait_ge(s_sum, r_sum)
                    sc.activation(e4[:], s4[:], mybir.ActivationFunctionType.Exp).then_inc(s_exp, 1)
                    sc.reg_add(r_aa, r_aa, 1)
                    sc.wait_ge(s_aa, r_aa)
                    sc.activation(gates[:, 0:384], a_sb[:, 0:384], mybir.ActivationFunctionType.Sigmoid)
                    sc.activation(gates[:, 384:512], a_sb[:, 384:512], mybir.ActivationFunctionType.Tanh).then_inc(s_sg, 1)
                    sc.reg_add(r_cu, r_cu, 1)
                    sc.wait_ge(s_cu, r_cu)
                    sc.activation(tanhc[:], ctile[:], mybir.ActivationFunctionType.Tanh).then_inc(s_th, 1)

    _es.close()
    lower_extended_insts(nc)
    return nc


_CACHE = {}


def _get_nc(t_steps):
    if t_steps not in _CACHE:
        _CACHE[t_steps] = build(t_steps)
    return _CACHE[t_steps]


def _chunked(w_cols):
    """[1024, 512] -> SBUF layout [128, 8*512] (h-chunk-major columns)."""
    return np.ascontiguousarray(
        w_cols.reshape(8, 128, w_cols.shape[1]).transpose(1, 0, 2).reshape(128, -1)
    )


def make_in_maps(x, A, Wx, Wh, Wattn, b, t_steps=T):
    x = np.asarray(x, dtype=np.float32)
    A = np.asarray(A, dtype=np.float32)
    Wx = np.asarray(Wx, dtype=np.float32)
    Wh = np.asarray(Wh, dtype=np.float32)
    Wattn = np.asarray(Wattn, dtype=np.float32)
    b = np.asarray(b, dtype=np.float32)

    A_flat = A.reshape(N, H, 16)
    # A4T[h, (g,q,nl,j)] = A_flat[32g+nl, h, 4q+j]
    arr = A_flat.reshape(4, 32, H, 4, 4)            # [g, nl, h, q, j]
    A4T = np.ascontiguousarray(arr.transpose(2, 0, 3, 1, 4).reshape(H, 2048))
    # Fold Wattn through A on the host: B_full[(g,q,nl,j), c] = sum_h A4T[h, row] Wattn[h, c]
    B_full = A4T.T @ Wattn                           # [2048, 4096] f32
    h0 = A_flat.mean(axis=2)                         # [N, H]

    # pre-transposed x: xtb[t, dc, 128*hc + n] = x[n, t, 128*hc + dc]
    xb = x[:, :t_steps, :].astype(BF)
    xtb = np.ascontiguousarray(
        xb.reshape(N, t_steps, 8, 128).transpose(1, 3, 2, 0).reshape(t_steps, 128, D)
    )

    part = np.arange(128)
    col = np.arange(512)
    mask = (part[:, None] // 4 == col[None, :] % 32)
    m512 = mask.astype(BF)
    dm512 = mask.astype(np.float32)
    ident = np.eye(128, dtype=np.float32)
    onesb = np.ones((128, 1), dtype=BF)

    in_maps = []
    for k in range(NCORES):
        cols = np.concatenate(
            [np.arange(g * H + 128 * k, g * H + 128 * k + 128) for g in range(4)]
        )
        in_maps.append({
            "xtb": xtb,
            "wh": _chunked(Wh[:, cols]).astype(BF),
            "wx": _chunked(Wx[:, cols]).astype(BF),
            "b4i": np.concatenate(
                [B_full[128 * blk:128 * (blk + 1), cols] for blk in range(16)],
                axis=1,
            ).astype(BF),
            "h0t": np.ascontiguousarray(h0[:, 128 * k:128 * (k + 1)].T),
            "c0i": np.ascontiguousarray(h0[:, 128 * k:128 * (k + 1)]),
            "a4tsc": (A4T[128 * k:128 * (k + 1), :] / 32.0).astype(BF),
            "btile": np.ascontiguousarray(
                np.broadcast_to(b[cols], (128, 512)).astype(np.float32)
            ),
            "m512": m512,
            "dm512": dm512,
            "ident": ident,
            "onesb": onesb,
            "cid": np.full((1, 1), k, np.int32),
        })
    return in_maps


LAST_TIMES = {}


def kernel(x, A, Wx, Wh, Wattn, b):
    import time as _time
    t_steps = x.shape[1]
    t0 = _time.monotonic()
    nc = _get_nc(t_steps)
    t1 = _time.monotonic()
    in_maps = make_in_maps(x, A, Wx, Wh, Wattn, b, t_steps)
    t2 = _time.monotonic()
    res = run_bass_kernel_spmd(nc, in_maps, core_ids=list(range(NCORES)))
    t3 = _time.monotonic()
    LAST_TIMES.update(build=t1 - t0, prep=t2 - t1, run=t3 - t2)
    out = np.empty((N, t_steps, H), dtype=np.float32)
    for k in range(NCORES):
        out[:, :, 128 * k:128 * (k + 1)] = res.results[k]["yout"]
    return out
